# revision 10
# baseline (speedup 1.0000x reference)
"""Transformer-XL relative-position multi-head attention on 8 Trainium2 cores.

Sharding: tensor-parallel over heads (16 heads -> 2 per core), data kept
full-batch on every core.  Each core computes its 2 heads' attention and a
partial output projection (Wo row-shard); the host sums the 8 partials.

Per-core dataflow (all layouts "transposed": d or j on partitions):
  qT/kT/rkT = W.T @ xT projections (f32r matmuls), v j-partitioned.
  BD term:   RELT[i,u] = (q+r_r_bias)[i] . rk[u]  (per i-tile, PSUM)
             -> bf16 -> DRAM bounce -> read back with a stride-(2047) flat
             access pattern (== Transformer-XL rel_shift restricted to the
             causal region) fused with an XBAR transpose -> BD^T[j,i] tiles.
  scores^T:  AC^T = k . (q+r_w_bias) matmul into PSUM, BD^T added via an
             identity matmul (PE cols are cheaper than DVE adds),
             ACT exp(scale=1/8) -> probs^T bf16 chunk, affine_select zeroes
             the causal boundary (which also kills rel-shift garbage).
  AV:        fused per (head, j-tile): probs^T chunk matmul with [v | 1]
             accumulates straight into two held PSUM banks (i in [0,512),
             [512,1024)); the ones column gives the softmax denominator.
             j-tiles processed descending so the bounce write->read chain
             pipelines behind the RELT matmuls.
"""

import sys

for _p in ("/opt/trn_rl_repo", "/root/.axon_site/_ro/trn_rl_repo"):
    if _p not in sys.path:
        sys.path.insert(0, _p)

import numpy as np
import ml_dtypes

import concourse.bass as bass
import concourse.bacc as bacc
import concourse.mybir as mybir
import concourse.tile as tile
from concourse.bass import MemorySpace
from concourse.masks import make_identity

# ---------------------------------------------------------------- constants
TOT_LEN, MEM_LEN, BSZ = 1024, 1024, 4
SEG_LEN = TOT_LEN + MEM_LEN          # 2048
D_EMBED, N_HEAD, D_HEAD = 1024, 16, 64
N_CORES = 8
NH_LOC = N_HEAD // N_CORES           # 2 heads per core
DH = NH_LOC * D_HEAD                 # 128
SCALE = 1.0 / (D_HEAD ** 0.5)        # 1/8

FP = mybir.dt.float32
F32R = mybir.dt.float32r
BF = mybir.dt.bfloat16

I_TILES = TOT_LEN // 128             # 8
J_TILES = SEG_LEN // 128             # 16
K_TILES = D_EMBED // 128             # 8

_PROGRAM = None


def _build_program(dbg=False, reps=1):
    """Build the SPMD per-core Bass program (identical on all 8 cores)."""
    nc = bacc.Bacc("TRN2", target_bir_lowering=False, debug=False)

    # DRAM I/O ------------------------------------------------------------
    catT = nc.dram_tensor("catT", [D_EMBED, BSZ, SEG_LEN], BF, kind="ExternalInput")
    rT = nc.dram_tensor("rT", [D_EMBED, SEG_LEN], BF, kind="ExternalInput")
    wq = nc.dram_tensor("wq", [D_EMBED, DH], BF, kind="ExternalInput")
    wk = nc.dram_tensor("wk", [D_EMBED, DH], BF, kind="ExternalInput")
    wv = nc.dram_tensor("wv", [D_EMBED, DH], BF, kind="ExternalInput")
    wr = nc.dram_tensor("wr", [D_EMBED, DH], BF, kind="ExternalInput")
    wo = nc.dram_tensor("wo", [DH, D_EMBED], F32R, kind="ExternalInput")
    bias_w = nc.dram_tensor("bias_w", [DH, 1], FP, kind="ExternalInput")
    bias_r = nc.dram_tensor("bias_r", [DH, 1], FP, kind="ExternalInput")
    out = nc.dram_tensor("out", [BSZ, TOT_LEN, D_EMBED], BF, kind="ExternalOutput")

    dbg_t = {}
    if dbg:
        dbg_t["qwT"] = nc.dram_tensor("dbg_qwT", [DH, TOT_LEN], BF, kind="ExternalOutput")
        dbg_t["qrT"] = nc.dram_tensor("dbg_qrT", [DH, TOT_LEN], BF, kind="ExternalOutput")
        dbg_t["kT"] = nc.dram_tensor("dbg_kT", [DH, SEG_LEN], BF, kind="ExternalOutput")
        dbg_t["rkT"] = nc.dram_tensor("dbg_rkT", [DH, SEG_LEN], BF, kind="ExternalOutput")
        dbg_t["v"] = nc.dram_tensor("dbg_v", [128, J_TILES, NH_LOC, D_HEAD + 1], BF, kind="ExternalOutput")
        dbg_t["avt"] = nc.dram_tensor("dbg_avt", [DH, TOT_LEN], F32R, kind="ExternalOutput")

    with tile.TileContext(nc) as tc:
        _emit(nc, tc, catT, rT, wq, wk, wv, wr, wo, bias_w, bias_r, out, dbg_t, reps)

    nc.compile()
    return nc


def _emit(nc, tc, catT, rT, wq, wk, wv, wr, wo, bias_w, bias_r, out, dbg_t={}, reps=1):
    from contextlib import ExitStack

    ctx = ExitStack()
    with ctx:
        consts = ctx.enter_context(tc.tile_pool(name="consts", bufs=1))
        stream = ctx.enter_context(tc.tile_pool(name="stream", bufs=6))
        projp = ctx.enter_context(tc.tile_pool(name="projp", bufs=3))
        reltp = ctx.enter_context(tc.tile_pool(name="reltp", bufs=6))
        bdtp = ctx.enter_context(tc.tile_pool(name="bdtp", bufs=6))
        probp = ctx.enter_context(tc.tile_pool(name="probp", bufs=6))
        avtp = ctx.enter_context(tc.tile_pool(name="avtp", bufs=2))
        outp = ctx.enter_context(tc.tile_pool(name="outp", bufs=4))
        smallp = ctx.enter_context(tc.tile_pool(name="smallp", bufs=2))
        psB = ctx.enter_context(tc.tile_pool(name="psB", bufs=3, space=MemorySpace.PSUM))
        dramp = ctx.enter_context(tc.tile_pool(name="dramp", bufs=4, space="DRAM"))

        # PSUM: 8 banks total = proj(3) + acps(3) + avps(2)
        def ps_tile(tag, bufs, name):
            return psB.tile([128, 512], FP, tag=tag, name=name, bufs=bufs)

        # ---------------- constants into SBUF ----------------
        def load_w(w_dram, name, eng):
            t = consts.tile([128, K_TILES, DH], BF, name=name, tag=name)
            eng.dma_start(t[:], w_dram.rearrange("(kt p) d -> p kt d", p=128))
            return t

        wq_sb = load_w(wq, "wq_sb", nc.scalar)
        wk_sb = load_w(wk, "wk_sb", nc.scalar)
        wv_sb = load_w(wv, "wv_sb", nc.scalar)
        wr_sb = load_w(wr, "wr_sb", nc.scalar)
        ident = consts.tile([128, 128], FP, name="ident", tag="ident")
        make_identity(nc, ident[:])
        ident_bf = consts.tile([128, 128], BF, name="ident_bf", tag="ident_bf")
        make_identity(nc, ident_bf[:])
        wo_sb = consts.tile([DH, D_EMBED], F32R)
        nc.scalar.dma_start(wo_sb[:], wo[:])
        bw_sb = consts.tile([DH, 1], FP)
        br_sb = consts.tile([DH, 1], FP)
        nc.scalar.dma_start(bw_sb[:], bias_w[:])
        nc.scalar.dma_start(br_sb[:], bias_r[:])

        # ---------------- rkT projection: (DH part, SEG_LEN) ----------------
        rkT_sb = consts.tile([DH, SEG_LEN], BF)
        for _rep in range(reps):
          for c0 in range(0, SEG_LEN, 512):
              ps = ps_tile("proj", 3, "rk_ps")
              rt_t = stream.tile([128, K_TILES, 512], BF, tag="rstream", name="rt_t", bufs=2)
              nc.scalar.dma_start(rt_t[:], rT.rearrange("(kt p) s -> p kt s", p=128)[:, :, c0:c0 + 512])
              for kt in range(K_TILES):
                  nc.tensor.matmul(ps[:], wr_sb[:, kt, :],
                                   rt_t[:, kt, :],
                                   start=(kt == 0), stop=(kt == K_TILES - 1))
              nc.vector.tensor_copy(rkT_sb[:, c0:c0 + 512], ps[:])

          # catT chunk loads: software-pipelined one batch ahead on the
          # Act/HWDGE queue so next batch's projections never wait on DMA
          CT_ORDER = (TOT_LEN, TOT_LEN + 512, 0, 512)
          cts = {}

          def load_ct(b):
              tiles = {}
              for c0 in CT_ORDER:
                  t = stream.tile([128, K_TILES, 512], BF, tag="instream",
                                  name="ct", bufs=6)
                  nc.scalar.dma_start(t[:], catT.rearrange(
                      "(kt p) b s -> p kt b s", p=128)[:, :, b, c0:c0 + 512])
                  tiles[c0] = t
              cts[b] = tiles

          load_ct(0)

          # ---------------- phase emitters ----------------
          def proj_phase(b):
              qwT = projp.tile([DH, TOT_LEN], BF, tag="qwT", name="qwT")
              qrT = projp.tile([DH, TOT_LEN], BF, tag="qrT", name="qrT")
              kT = projp.tile([DH, SEG_LEN], BF, tag="kT", name="kT")
              # v: j-partitioned, per (j-tile, head): (128, jt, n, 65); col 64 = ones
              v_sb = projp.tile([128, J_TILES, NH_LOC, D_HEAD + 1], BF, tag="v", name="v_sb")
              nc.vector.memset(v_sb[:, :, :, 64], 1.0)

              for c0 in CT_ORDER:
                  ct = cts[b][c0]
                  kps = ps_tile("proj", 3, "kps")
                  for kt in range(K_TILES):
                      nc.tensor.matmul(kps[:], wk_sb[:, kt, :], ct[:, kt, :],
                                       start=(kt == 0), stop=(kt == K_TILES - 1))
                  nc.vector.tensor_copy(kT[:, c0:c0 + 512], kps[:])
                  if c0 >= TOT_LEN:
                      qps = ps_tile("proj", 3, "qps")
                      for kt in range(K_TILES):
                          nc.tensor.matmul(qps[:], wq_sb[:, kt, :], ct[:, kt, :],
                                           start=(kt == 0), stop=(kt == K_TILES - 1))
                      i0c = c0 - TOT_LEN
                      nc.vector.tensor_scalar_add(qwT[:, i0c:i0c + 512], qps[:], bw_sb[:])
                      nc.vector.tensor_scalar_add(qrT[:, i0c:i0c + 512], qps[:], br_sb[:])
                  vps = ps_tile("proj", 3, "vps")  # vT chunk: (128 dh, 512 j)
                  for kt in range(K_TILES):
                      nc.tensor.matmul(vps[:], wv_sb[:, kt, :], ct[:, kt, :],
                                       start=(kt == 0), stop=(kt == K_TILES - 1))
                  # transpose vT chunk -> v (j-partitioned) via PE
                  vtc = stream.tile([128, 512], FP, tag="vtc", name="vtc", bufs=4)
                  nc.vector.tensor_copy(vtc[:], vps[:])
                  vtp = ps_tile("proj", 3, "vtp")
                  for jj in range(4):
                      jt = c0 // 128 + jj
                      nc.tensor.transpose(vtp[:, jj * 128:(jj + 1) * 128],
                                          vtc[:, jj * 128:(jj + 1) * 128], ident[:])
                      nc.vector.tensor_copy(v_sb[:, jt, :, 0:64],
                                            vtp[:, jj * 128:(jj + 1) * 128].rearrange("p (n d) -> p n d", n=NH_LOC))

              if dbg_t and b == 0:
                  nc.scalar.dma_start(dbg_t["qwT"][:], qwT[:])
                  nc.scalar.dma_start(dbg_t["qrT"][:], qrT[:])
                  nc.scalar.dma_start(dbg_t["kT"][:], kT[:])
                  nc.scalar.dma_start(dbg_t["rkT"][:], rkT_sb[:])
                  nc.scalar.dma_start(dbg_t["v"][:], v_sb[:])
              return qwT, qrT, kT, v_sb

          # ---- 2a: RELT per i-tile -> bf16 -> DRAM bounce ----
          # descending i-tiles: the widest window (it=7, needed by every
          # j-tile read) is written first, so descending-jt score reads
          # can start while later RELT tiles still compute
          def relt_phase(b, qrT):
              bounces = []
              for n in range(NH_LOC):
                  p_lo, p_hi = n * 64, (n + 1) * 64
                  bounce = dramp.tile([TOT_LEN, SEG_LEN], BF, name=f"bounce{n}")
                  bounces.append(bounce)
                  for it in reversed(range(I_TILES)):
                      i0 = it * 128
                      u_lo = (TOT_LEN - 128) - i0          # 896 - i0
                      relt_sb = reltp.tile([128, SEG_LEN], BF, tag="relt", name="relt_sb")
                      for ci, c0 in enumerate(range(u_lo, SEG_LEN, 512)):
                          cw = min(512, SEG_LEN - c0)
                          rps = ps_tile("proj", 3, "rps")
                          nc.tensor.matmul(rps[:, 0:cw],
                                           qrT[p_lo:p_hi, i0:i0 + 128],
                                           rkT_sb[p_lo:p_hi, c0:c0 + cw],
                                           start=True, stop=True)
                          if ci % 3 == 0:
                              nc.scalar.copy(relt_sb[:, c0:c0 + cw], rps[:, 0:cw])
                          else:
                              nc.vector.tensor_copy(relt_sb[:, c0:c0 + cw], rps[:, 0:cw])
                      # Act/HWDGE queue: cheap trigger, and a different hw DMA
                      # queue than the SP-issued shear reads, so next batch's
                      # writes can't head-of-line-block this batch's reads
                      nc.scalar.dma_start(bounce[i0:i0 + 128, u_lo:SEG_LEN],
                                          relt_sb[:, u_lo:SEG_LEN])
              return bounces

          # ---- 2b: fused scores + AV per head, descending j-tiles ----
          def attn_phase(b, qwT, kT, v_sb, bounces):
              # AV^T accumulator for both heads, f32: (128 = n*64+d, TOT_LEN)
              avt_sb = avtp.tile([DH, TOT_LEN], F32R, tag="avt", name="avt_sb")
              for n in range(NH_LOC):
                  p_lo, p_hi = n * 64, (n + 1) * 64
                  bflat = bounces[n][:]
                  avA = ps_tile("avps", 2, "avA")   # i in [0, 512)
                  avB = ps_tile("avps", 2, "avB")   # i in [512, 1024)
                  for jt in reversed(range(J_TILES)):
                      j0 = jt * 128
                      i_start = max(0, j0 - MEM_LEN)
                      iw = TOT_LEN - i_start
                      diag = j0 >= MEM_LEN
                      # shifted + transposed read of the bounce buffer:
                      # BD[i, j] = RELT[i, j + 1023 - i] == flat[i*2047 + j + 1023]
                      bdt = bdtp.tile([128, TOT_LEN], BF, tag="bdt", name="bdt")
                      src = bass.AP(
                          tensor=bflat.tensor,
                          offset=bflat.offset + i_start * (SEG_LEN - 1) + j0 + (TOT_LEN - 1),
                          ap=[[SEG_LEN - 1, iw], [1, 128]],
                      )
                      nc.sync.dma_start(bdt[:, 0:iw], src, transpose=True)
                      if diag:
                          # zero rel-shift garbage (j > i + MEM_LEN) so the
                          # identity-matmul add can't smear NaNs across columns
                          nc.gpsimd.affine_select(
                              out=bdt[:, 0:128], in_=bdt[:, 0:128],
                              compare_op=mybir.AluOpType.is_ge,
                              fill=0.0, base=0, channel_multiplier=-1,
                              pattern=[[1, 128]],
                          )
                      # sub-chunks: (bdt col offset, i_lo, width, bank)
                      if i_start < 512:
                          chunks = [(0, i_start, 512 - i_start, 0),
                                    (512 - i_start, 512, 512, 1)]
                      else:
                          chunks = [(0, i_start, TOT_LEN - i_start, 1)]
                      acps = [ps_tile("acps", 3, "acps") for _ in chunks]
                      for (boff, ilo, cw, bank), a in zip(chunks, acps):
                          nc.tensor.matmul(a[:, 0:cw], ident_bf[:],
                                           bdt[:, boff:boff + cw],
                                           start=True, stop=False)
                      for (boff, ilo, cw, bank), a in zip(chunks, acps):
                          nc.tensor.matmul(a[:, 0:cw],
                                           kT[p_lo:p_hi, j0:j0 + 128],
                                           qwT[p_lo:p_hi, ilo:ilo + cw],
                                           start=False, stop=True)
                      # first touch of a bank (descending jt) covers only a
                      # partial column range; pad the probs tile with zeros
                      # and run a full-width start=True matmul so the whole
                      # PSUM bank is initialized
                      probs, poffs = [], []
                      for (boff, ilo, cw, bank), a in zip(chunks, acps):
                          st = (jt == 11) if bank == 0 else (jt == J_TILES - 1)
                          poff = ilo - bank * 512 if st else 0
                          p = probp.tile([128, 512], BF, tag="probs", name="probs")
                          if st and poff > 0:
                              nc.vector.memset(p[:, 0:poff], 0.0)
                          nc.scalar.activation(p[:, poff:poff + cw], a[:, 0:cw],
                                               mybir.ActivationFunctionType.Exp,
                                               scale=SCALE)
                          probs.append(p)
                          poffs.append(poff)
                      if diag:
                          # zero where j > i + MEM_LEN: keep where y - jp >= 0
                          nc.gpsimd.affine_select(
                              out=probs[0][:, poffs[0]:poffs[0] + 128],
                              in_=probs[0][:, poffs[0]:poffs[0] + 128],
                              compare_op=mybir.AluOpType.is_ge,
                              fill=0.0, base=0, channel_multiplier=-1,
                              pattern=[[1, 128]],
                          )
                      for (boff, ilo, cw, bank), p, poff in zip(chunks, probs, poffs):
                          av = avB if bank else avA
                          st = (jt == 11) if bank == 0 else (jt == J_TILES - 1)
                          if st:
                              nc.tensor.matmul(av[0:D_HEAD + 1, 0:512],
                                               v_sb[:, jt, n, :], p[:, 0:512],
                                               start=True, stop=(jt == 0))
                          else:
                              nc.tensor.matmul(av[0:D_HEAD + 1, ilo - bank * 512:ilo - bank * 512 + cw],
                                               v_sb[:, jt, n, :], p[:, 0:cw],
                                               start=False, stop=(jt == 0))
                  # ---- normalize both banks -> avt ----
                  for bank, av in ((0, avA), (1, avB)):
                      c0 = bank * 512
                      recip = smallp.tile([1, 512], FP, tag="recip", name="recip")
                      rbc = smallp.tile([64, 512], FP, tag="rbc", name="rbc")
                      nc.vector.reciprocal(recip[:], av[64:65, :])
                      nc.gpsimd.partition_broadcast(rbc[:], recip[:])
                      nc.vector.tensor_mul(avt_sb[p_lo:p_hi, c0:c0 + 512],
                                           av[0:64, :], rbc[:])

              if dbg_t and b == 0:
                  nc.scalar.dma_start(dbg_t["avt"][:], avt_sb[:])

              # ---- 3: partial output projection for batch b ----
              for it in range(I_TILES):
                  i0 = it * 128
                  for ec in range(2):
                      ops = ps_tile("avps", 2, "ops")
                      nc.tensor.matmul(ops[:],
                                       avt_sb[:, i0:i0 + 128],
                                       wo_sb[:, ec * 512:(ec + 1) * 512],
                                       start=True, stop=True)
                      ot = outp.tile([128, 512], BF, tag="ot", name="ot")
                      nc.vector.tensor_copy(ot[:], ops[:])
                      nc.scalar.dma_start(out[b, i0:i0 + 128, ec * 512:(ec + 1) * 512], ot[:])

          # ---------------- batch loop, software-pipelined ----------------
          # relt+bounce writes for batch b are issued a full phase before the
          # scores of batch b run (during which batch b-1's scores execute),
          # hiding the bounce write->shear read DMA latency entirely
          prev = None
          for b in range(BSZ):
              qwT, qrT, kT, v_sb = proj_phase(b)
              if b + 1 < BSZ:
                  load_ct(b + 1)
              bounces = relt_phase(b, qrT)
              if prev is not None:
                  attn_phase(*prev)
              prev = (b, qwT, kT, v_sb, bounces)
          attn_phase(*prev)


def _get_program():
    global _PROGRAM
    if _PROGRAM is None:
        _PROGRAM = _build_program()
    return _PROGRAM


def _prep_inputs(w, r, r_w_bias, r_r_bias, attn_mask, mems, Wqkv, Wr, Wo):
    """Host-side sharding: returns list of 8 per-core input dicts."""
    bf16 = ml_dtypes.bfloat16
    cat = np.concatenate([mems, w], axis=0)               # (S, b, E)
    catT = np.ascontiguousarray(cat.transpose(2, 1, 0)).astype(bf16)  # (E, b, S)
    rT = np.ascontiguousarray(r.T).astype(bf16)           # (E, S)

    in_maps = []
    for core in range(N_CORES):
        n0 = core * NH_LOC
        cs, ce = n0 * D_HEAD, (n0 + NH_LOC) * D_HEAD
        in_maps.append({
            "catT": catT,
            "rT": rT,
            "wq": np.ascontiguousarray(Wqkv[:, cs:ce]).astype(bf16),
            "wk": np.ascontiguousarray(Wqkv[:, D_EMBED + cs:D_EMBED + ce]).astype(bf16),
            "wv": np.ascontiguousarray(Wqkv[:, 2 * D_EMBED + cs:2 * D_EMBED + ce]).astype(bf16),
            "wr": np.ascontiguousarray(Wr[:, cs:ce]).astype(bf16),
            "wo": np.ascontiguousarray(Wo[cs:ce, :]),
            "bias_w": np.ascontiguousarray(r_w_bias[n0:n0 + NH_LOC].reshape(DH, 1)),
            "bias_r": np.ascontiguousarray(r_r_bias[n0:n0 + NH_LOC].reshape(DH, 1)),
        })
    return in_maps


def kernel(w, r, r_w_bias, r_r_bias, attn_mask, mems, Wqkv, Wr, Wo):
    from concourse.bass_utils import run_bass_kernel_spmd

    nc = _get_program()
    in_maps = _prep_inputs(w, r, r_w_bias, r_r_bias, attn_mask, mems, Wqkv, Wr, Wo)
    res = run_bass_kernel_spmd(nc, in_maps, list(range(N_CORES)))
    # out per core: (b, i, e) bf16 partial; sum over cores (head groups)
    total = np.zeros((BSZ, TOT_LEN, D_EMBED), np.float32)
    for core in range(N_CORES):
        total += res.results[core]["out"].astype(np.float32)
    return np.ascontiguousarray(total.transpose(1, 0, 2))  # (i, b, e)


# revision 15
# speedup vs baseline: 1.7735x; 1.7735x over previous
"""Transformer-XL relative-position multi-head attention on 8 Trainium2 cores.

Sharding: tensor-parallel over heads (16 heads -> 2 per core), data kept
full-batch on every core.  Each core computes its 2 heads' attention and a
partial output projection (Wo row-shard); the host sums the 8 partials.

Per-core dataflow (all layouts "transposed": d or j on partitions):
  qT/kT/rkT = W.T @ xT projections (f32r matmuls), v j-partitioned.
  BD term:   RELT[i,u] = (q+r_r_bias)[i] . rk[u]  (per i-tile, PSUM)
             -> bf16 -> DRAM bounce -> read back with a stride-(2047) flat
             access pattern (== Transformer-XL rel_shift restricted to the
             causal region) fused with an XBAR transpose -> BD^T[j,i] tiles.
  scores^T:  AC^T = k . (q+r_w_bias) matmul into PSUM, BD^T added via an
             identity matmul (PE cols are cheaper than DVE adds),
             ACT exp(scale=1/8) -> probs^T bf16 chunk, affine_select zeroes
             the causal boundary (which also kills rel-shift garbage).
  AV:        fused per (head, j-tile): probs^T chunk matmul with [v | 1]
             accumulates straight into two held PSUM banks (i in [0,512),
             [512,1024)); the ones column gives the softmax denominator.
             j-tiles processed descending so the bounce write->read chain
             pipelines behind the RELT matmuls.
"""

import sys

for _p in ("/opt/trn_rl_repo", "/root/.axon_site/_ro/trn_rl_repo"):
    if _p not in sys.path:
        sys.path.insert(0, _p)

import numpy as np
import ml_dtypes

import concourse.bass as bass
import concourse.bacc as bacc
import concourse.mybir as mybir
import concourse.tile as tile
from concourse.bass import MemorySpace
from concourse.masks import make_identity

# ---------------------------------------------------------------- constants
TOT_LEN, MEM_LEN, BSZ = 1024, 1024, 4
SEG_LEN = TOT_LEN + MEM_LEN          # 2048
D_EMBED, N_HEAD, D_HEAD = 1024, 16, 64
N_CORES = 8
NH_LOC = N_HEAD // N_CORES           # 2 heads per core
DH = NH_LOC * D_HEAD                 # 128
SCALE = 1.0 / (D_HEAD ** 0.5)        # 1/8

FP = mybir.dt.float32
F32R = mybir.dt.float32r
BF = mybir.dt.bfloat16

I_TILES = TOT_LEN // 128             # 8
J_TILES = SEG_LEN // 128             # 16
K_TILES = D_EMBED // 128             # 8

_PROGRAM = None


def _build_program(dbg=False, reps=1):
    """Build the SPMD per-core Bass program (identical on all 8 cores)."""
    nc = bacc.Bacc("TRN2", target_bir_lowering=False, debug=False)

    # DRAM I/O ------------------------------------------------------------
    catT = nc.dram_tensor("catT", [D_EMBED, BSZ, SEG_LEN], BF, kind="ExternalInput")
    rT = nc.dram_tensor("rT", [D_EMBED, SEG_LEN], BF, kind="ExternalInput")
    wq = nc.dram_tensor("wq", [D_EMBED, DH], BF, kind="ExternalInput")
    wk = nc.dram_tensor("wk", [D_EMBED, DH], BF, kind="ExternalInput")
    wv = nc.dram_tensor("wv", [D_EMBED, DH], BF, kind="ExternalInput")
    wr = nc.dram_tensor("wr", [D_EMBED, DH], BF, kind="ExternalInput")
    wo = nc.dram_tensor("wo", [DH, D_EMBED], F32R, kind="ExternalInput")
    bias_w = nc.dram_tensor("bias_w", [DH, 1], FP, kind="ExternalInput")
    bias_r = nc.dram_tensor("bias_r", [DH, 1], FP, kind="ExternalInput")
    out = nc.dram_tensor("out", [BSZ, TOT_LEN, D_EMBED], BF, kind="ExternalOutput")

    dbg_t = {}
    if dbg:
        dbg_t["qwT"] = nc.dram_tensor("dbg_qwT", [DH, TOT_LEN], BF, kind="ExternalOutput")
        dbg_t["qrT"] = nc.dram_tensor("dbg_qrT", [DH, TOT_LEN], BF, kind="ExternalOutput")
        dbg_t["kT"] = nc.dram_tensor("dbg_kT", [DH, SEG_LEN], BF, kind="ExternalOutput")
        dbg_t["rkT"] = nc.dram_tensor("dbg_rkT", [DH, SEG_LEN], BF, kind="ExternalOutput")
        dbg_t["v"] = nc.dram_tensor("dbg_v", [128, J_TILES, NH_LOC, D_HEAD + 1], BF, kind="ExternalOutput")
        dbg_t["avt"] = nc.dram_tensor("dbg_avt", [DH, TOT_LEN], F32R, kind="ExternalOutput")

    with tile.TileContext(nc) as tc:
        _emit(nc, tc, catT, rT, wq, wk, wv, wr, wo, bias_w, bias_r, out, dbg_t, reps)

    nc.compile()
    return nc


def _emit(nc, tc, catT, rT, wq, wk, wv, wr, wo, bias_w, bias_r, out, dbg_t={}, reps=1):
    from contextlib import ExitStack

    ctx = ExitStack()
    with ctx:
        consts = ctx.enter_context(tc.tile_pool(name="consts", bufs=1))
        stream = ctx.enter_context(tc.tile_pool(name="stream", bufs=6))
        projp = ctx.enter_context(tc.tile_pool(name="projp", bufs=3))
        reltp = ctx.enter_context(tc.tile_pool(name="reltp", bufs=6))
        bdtp = ctx.enter_context(tc.tile_pool(name="bdtp", bufs=6))
        probp = ctx.enter_context(tc.tile_pool(name="probp", bufs=10))
        avtp = ctx.enter_context(tc.tile_pool(name="avtp", bufs=2))
        outp = ctx.enter_context(tc.tile_pool(name="outp", bufs=4))
        smallp = ctx.enter_context(tc.tile_pool(name="smallp", bufs=2))
        psB = ctx.enter_context(tc.tile_pool(name="psB", bufs=3, space=MemorySpace.PSUM))
        dramp = ctx.enter_context(tc.tile_pool(name="dramp", bufs=4, space="DRAM"))

        # PSUM: 8 banks total = proj(3) + acps(3) + avps(2)
        def ps_tile(tag, bufs, name):
            return psB.tile([128, 512], FP, tag=tag, name=name, bufs=bufs)

        # ---------------- constants into SBUF ----------------
        def load_w(w_dram, name, eng):
            t = consts.tile([128, K_TILES, DH], BF, name=name, tag=name)
            eng.dma_start(t[:], w_dram.rearrange("(kt p) d -> p kt d", p=128))
            return t

        wq_sb = load_w(wq, "wq_sb", nc.scalar)
        wk_sb = load_w(wk, "wk_sb", nc.scalar)
        wv_sb = load_w(wv, "wv_sb", nc.scalar)
        wr_sb = load_w(wr, "wr_sb", nc.scalar)
        ident = consts.tile([128, 128], FP, name="ident", tag="ident")
        make_identity(nc, ident[:])
        ident_bf = consts.tile([128, 128], BF, name="ident_bf", tag="ident_bf")
        make_identity(nc, ident_bf[:])
        wo_sb = consts.tile([DH, D_EMBED], F32R)
        nc.scalar.dma_start(wo_sb[:], wo[:])
        bw_sb = consts.tile([DH, 1], FP)
        br_sb = consts.tile([DH, 1], FP)
        nc.scalar.dma_start(bw_sb[:], bias_w[:])
        nc.scalar.dma_start(br_sb[:], bias_r[:])

        # ---------------- rkT projection: (DH part, SEG_LEN) ----------------
        rkT_sb = consts.tile([DH, SEG_LEN], BF)
        for _rep in range(reps):
          for c0 in range(0, SEG_LEN, 512):
              ps = ps_tile("proj", 3, "rk_ps")
              rt_t = stream.tile([128, K_TILES, 512], BF, tag="rstream", name="rt_t", bufs=2)
              nc.scalar.dma_start(rt_t[:], rT.rearrange("(kt p) s -> p kt s", p=128)[:, :, c0:c0 + 512])
              for kt in range(K_TILES):
                  nc.tensor.matmul(ps[:], wr_sb[:, kt, :],
                                   rt_t[:, kt, :],
                                   start=(kt == 0), stop=(kt == K_TILES - 1))
              nc.vector.tensor_copy(rkT_sb[:, c0:c0 + 512], ps[:])

          # catT chunk loads: software-pipelined one batch ahead on the
          # Act/HWDGE queue so next batch's projections never wait on DMA
          CT_ORDER = (TOT_LEN, TOT_LEN + 512, 0, 512)
          cts = {}

          def load_ct(b):
              tiles = {}
              for c0 in CT_ORDER:
                  t = stream.tile([128, K_TILES, 512], BF, tag="instream",
                                  name="ct", bufs=6)
                  # Pool/SWDGE ring: its own hw DMA ring (not behind the
                  # bounce writes on the Act ring), and triggers fire during
                  # the proj phase when the Pool queue is otherwise idle
                  nc.gpsimd.dma_start(t[:], catT.rearrange(
                      "(kt p) b s -> p kt b s", p=128)[:, :, b, c0:c0 + 512])
                  tiles[c0] = t
              cts[b] = tiles

          load_ct(0)

          # ---------------- phase emitters ----------------
          def proj_phase(b):
              qwT = projp.tile([DH, TOT_LEN], BF, tag="qwT", name="qwT")
              qrT = projp.tile([DH, TOT_LEN], BF, tag="qrT", name="qrT")
              kT = projp.tile([DH, SEG_LEN], BF, tag="kT", name="kT")
              # v: j-partitioned, per (j-tile, head): (128, jt, n, 65); col 64 = ones
              v_sb = projp.tile([128, J_TILES, NH_LOC, D_HEAD + 1], BF, tag="v", name="v_sb")
              nc.vector.memset(v_sb[:, :, :, 64], 1.0)

              for c0 in CT_ORDER:
                  ct = cts[b][c0]
                  kps = ps_tile("proj", 3, "kps")
                  for kt in range(K_TILES):
                      nc.tensor.matmul(kps[:], wk_sb[:, kt, :], ct[:, kt, :],
                                       start=(kt == 0), stop=(kt == K_TILES - 1))
                  nc.vector.tensor_copy(kT[:, c0:c0 + 512], kps[:])
                  if c0 >= TOT_LEN:
                      qps = ps_tile("proj", 3, "qps")
                      for kt in range(K_TILES):
                          nc.tensor.matmul(qps[:], wq_sb[:, kt, :], ct[:, kt, :],
                                           start=(kt == 0), stop=(kt == K_TILES - 1))
                      i0c = c0 - TOT_LEN
                      nc.vector.tensor_scalar_add(qwT[:, i0c:i0c + 512], qps[:], bw_sb[:])
                      nc.vector.tensor_scalar_add(qrT[:, i0c:i0c + 512], qps[:], br_sb[:])
                  vps = ps_tile("proj", 3, "vps")  # vT chunk: (128 dh, 512 j)
                  for kt in range(K_TILES):
                      nc.tensor.matmul(vps[:], wv_sb[:, kt, :], ct[:, kt, :],
                                       start=(kt == 0), stop=(kt == K_TILES - 1))
                  # transpose vT chunk -> v (j-partitioned) via PE
                  vtc = stream.tile([128, 512], FP, tag="vtc", name="vtc", bufs=4)
                  nc.vector.tensor_copy(vtc[:], vps[:])
                  vtp = ps_tile("proj", 3, "vtp")
                  for jj in range(4):
                      jt = c0 // 128 + jj
                      nc.tensor.transpose(vtp[:, jj * 128:(jj + 1) * 128],
                                          vtc[:, jj * 128:(jj + 1) * 128], ident[:])
                      nc.vector.tensor_copy(v_sb[:, jt, :, 0:64],
                                            vtp[:, jj * 128:(jj + 1) * 128].rearrange("p (n d) -> p n d", n=NH_LOC))

              if dbg_t and b == 0:
                  nc.scalar.dma_start(dbg_t["qwT"][:], qwT[:])
                  nc.scalar.dma_start(dbg_t["qrT"][:], qrT[:])
                  nc.scalar.dma_start(dbg_t["kT"][:], kT[:])
                  nc.scalar.dma_start(dbg_t["rkT"][:], rkT_sb[:])
                  nc.scalar.dma_start(dbg_t["v"][:], v_sb[:])
              return qwT, qrT, kT, v_sb

          # ---- 2a: RELT per i-tile -> bf16 -> DRAM bounce ----
          # descending i-tiles: the widest window (it=7, needed by every
          # j-tile read) is written first, so descending-jt score reads
          # can start while later RELT tiles still compute
          def relt_phase(b, qrT):
              bounces = []
              for n in range(NH_LOC):
                  p_lo, p_hi = n * 64, (n + 1) * 64
                  bounce = dramp.tile([TOT_LEN, SEG_LEN], BF, name=f"bounce{n}")
                  bounces.append(bounce)
                  for it in reversed(range(I_TILES)):
                      i0 = it * 128
                      u_lo = (TOT_LEN - 128) - i0          # 896 - i0
                      relt_sb = reltp.tile([128, SEG_LEN], BF, tag="relt", name="relt_sb")
                      for ci, c0 in enumerate(range(u_lo, SEG_LEN, 512)):
                          cw = min(512, SEG_LEN - c0)
                          rps = ps_tile("proj", 3, "rps")
                          nc.tensor.matmul(rps[:, 0:cw],
                                           qrT[p_lo:p_hi, i0:i0 + 128],
                                           rkT_sb[p_lo:p_hi, c0:c0 + cw],
                                           start=True, stop=True)
                          if ci % 3 == 0:
                              nc.scalar.copy(relt_sb[:, c0:c0 + cw], rps[:, 0:cw])
                          else:
                              nc.vector.tensor_copy(relt_sb[:, c0:c0 + cw], rps[:, 0:cw])
                      # Act/HWDGE queue: cheap trigger, and a different hw DMA
                      # queue than the SP-issued shear reads, so next batch's
                      # writes can't head-of-line-block this batch's reads
                      nc.scalar.dma_start(bounce[i0:i0 + 128, u_lo:SEG_LEN],
                                          relt_sb[:, u_lo:SEG_LEN])
              return bounces

          # ---- 2b: fused scores + AV per head, descending j-tiles ----
          def attn_phase(b, qwT, kT, v_sb, bounces):
              # AV^T accumulator for both heads, f32: (128 = n*64+d, TOT_LEN)
              avt_sb = avtp.tile([DH, TOT_LEN], F32R, tag="avt", name="avt_sb")
              AV_LAG = 3  # AV(jt) emitted AV_LAG j-tiles after its exp, so the
              #             PE never waits on a fresh Activation round-trip
              for n in range(NH_LOC):
                  p_lo, p_hi = n * 64, (n + 1) * 64
                  bflat = bounces[n][:]
                  avA = ps_tile("avps", 2, "avA")   # i in [0, 512)
                  avB = ps_tile("avps", 2, "avB")   # i in [512, 1024)

                  def emit_av(jt, chunks, probs, poffs):
                      for (boff, ilo, cw, bank), p, poff in zip(chunks, probs, poffs):
                          av = avB if bank else avA
                          st = (jt == 11) if bank == 0 else (jt == J_TILES - 1)
                          if st:
                              nc.tensor.matmul(av[0:D_HEAD + 1, 0:512],
                                               v_sb[:, jt, n, :], p[:, 0:512],
                                               start=True, stop=(jt == 0))
                          else:
                              nc.tensor.matmul(av[0:D_HEAD + 1, ilo - bank * 512:ilo - bank * 512 + cw],
                                               v_sb[:, jt, n, :], p[:, 0:cw],
                                               start=False, stop=(jt == 0))

                  pending = []
                  for jt in reversed(range(J_TILES)):
                      j0 = jt * 128
                      i_start = max(0, j0 - MEM_LEN)
                      iw = TOT_LEN - i_start
                      diag = j0 >= MEM_LEN
                      # shifted + transposed read of the bounce buffer:
                      # BD[i, j] = RELT[i, j + 1023 - i] == flat[i*2047 + j + 1023]
                      bdt = bdtp.tile([128, TOT_LEN], BF, tag="bdt", name="bdt")
                      src = bass.AP(
                          tensor=bflat.tensor,
                          offset=bflat.offset + i_start * (SEG_LEN - 1) + j0 + (TOT_LEN - 1),
                          ap=[[SEG_LEN - 1, iw], [1, 128]],
                      )
                      nc.sync.dma_start(bdt[:, 0:iw], src, transpose=True)
                      if diag:
                          # zero rel-shift garbage (j > i + MEM_LEN) so the
                          # identity-matmul add can't smear NaNs across columns
                          nc.gpsimd.affine_select(
                              out=bdt[:, 0:128], in_=bdt[:, 0:128],
                              compare_op=mybir.AluOpType.is_ge,
                              fill=0.0, base=0, channel_multiplier=-1,
                              pattern=[[1, 128]],
                          )
                      # sub-chunks: (bdt col offset, i_lo, width, bank)
                      if i_start < 512:
                          chunks = [(0, i_start, 512 - i_start, 0),
                                    (512 - i_start, 512, 512, 1)]
                      else:
                          chunks = [(0, i_start, TOT_LEN - i_start, 1)]
                      acps = [ps_tile("acps", 3, "acps") for _ in chunks]
                      for (boff, ilo, cw, bank), a in zip(chunks, acps):
                          nc.tensor.matmul(a[:, 0:cw], ident_bf[:],
                                           bdt[:, boff:boff + cw],
                                           start=True, stop=False)
                      for (boff, ilo, cw, bank), a in zip(chunks, acps):
                          nc.tensor.matmul(a[:, 0:cw],
                                           kT[p_lo:p_hi, j0:j0 + 128],
                                           qwT[p_lo:p_hi, ilo:ilo + cw],
                                           start=False, stop=True)
                      # first touch of a bank (descending jt) covers only a
                      # partial column range; pad the probs tile with zeros
                      # and run a full-width start=True matmul so the whole
                      # PSUM bank is initialized
                      probs, poffs = [], []
                      for (boff, ilo, cw, bank), a in zip(chunks, acps):
                          st = (jt == 11) if bank == 0 else (jt == J_TILES - 1)
                          poff = ilo - bank * 512 if st else 0
                          p = probp.tile([128, 512], BF, tag="probs", name="probs")
                          if st and poff > 0:
                              nc.vector.memset(p[:, 0:poff], 0.0)
                          nc.scalar.activation(p[:, poff:poff + cw], a[:, 0:cw],
                                               mybir.ActivationFunctionType.Exp,
                                               scale=SCALE)
                          probs.append(p)
                          poffs.append(poff)
                      if diag:
                          # zero where j > i + MEM_LEN: keep where y - jp >= 0
                          nc.gpsimd.affine_select(
                              out=probs[0][:, poffs[0]:poffs[0] + 128],
                              in_=probs[0][:, poffs[0]:poffs[0] + 128],
                              compare_op=mybir.AluOpType.is_ge,
                              fill=0.0, base=0, channel_multiplier=-1,
                              pattern=[[1, 128]],
                          )
                      pending.append((jt, chunks, probs, poffs))
                      if len(pending) > AV_LAG:
                          emit_av(*pending.pop(0))
                  for item in pending:
                      emit_av(*item)
                  # ---- normalize both banks -> avt ----
                  for bank, av in ((0, avA), (1, avB)):
                      c0 = bank * 512
                      recip = smallp.tile([1, 512], FP, tag="recip", name="recip")
                      rbc = smallp.tile([64, 512], FP, tag="rbc", name="rbc")
                      nc.vector.reciprocal(recip[:], av[64:65, :])
                      nc.gpsimd.partition_broadcast(rbc[:], recip[:])
                      nc.vector.tensor_mul(avt_sb[p_lo:p_hi, c0:c0 + 512],
                                           av[0:64, :], rbc[:])

              if dbg_t and b == 0:
                  nc.scalar.dma_start(dbg_t["avt"][:], avt_sb[:])

              # ---- 3: partial output projection for batch b ----
              for it in range(I_TILES):
                  i0 = it * 128
                  for ec in range(2):
                      ops = ps_tile("avps", 2, "ops")
                      nc.tensor.matmul(ops[:],
                                       avt_sb[:, i0:i0 + 128],
                                       wo_sb[:, ec * 512:(ec + 1) * 512],
                                       start=True, stop=True)
                      ot = outp.tile([128, 512], BF, tag="ot", name="ot")
                      nc.vector.tensor_copy(ot[:], ops[:])
                      nc.scalar.dma_start(out[b, i0:i0 + 128, ec * 512:(ec + 1) * 512], ot[:])

          # ---------------- batch loop, software-pipelined ----------------
          # relt+bounce writes for batch b are issued a full phase before the
          # scores of batch b run (during which batch b-1's scores execute),
          # hiding the bounce write->shear read DMA latency entirely
          prev = None
          for b in range(BSZ):
              qwT, qrT, kT, v_sb = proj_phase(b)
              if b + 1 < BSZ:
                  load_ct(b + 1)
              bounces = relt_phase(b, qrT)
              if prev is not None:
                  attn_phase(*prev)
              prev = (b, qwT, kT, v_sb, bounces)
          attn_phase(*prev)


def _get_program():
    global _PROGRAM
    if _PROGRAM is None:
        _PROGRAM = _build_program()
    return _PROGRAM


def _prep_inputs(w, r, r_w_bias, r_r_bias, attn_mask, mems, Wqkv, Wr, Wo):
    """Host-side sharding: returns list of 8 per-core input dicts."""
    bf16 = ml_dtypes.bfloat16
    cat = np.concatenate([mems, w], axis=0)               # (S, b, E)
    catT = np.ascontiguousarray(cat.transpose(2, 1, 0)).astype(bf16)  # (E, b, S)
    rT = np.ascontiguousarray(r.T).astype(bf16)           # (E, S)

    in_maps = []
    for core in range(N_CORES):
        n0 = core * NH_LOC
        cs, ce = n0 * D_HEAD, (n0 + NH_LOC) * D_HEAD
        in_maps.append({
            "catT": catT,
            "rT": rT,
            "wq": np.ascontiguousarray(Wqkv[:, cs:ce]).astype(bf16),
            "wk": np.ascontiguousarray(Wqkv[:, D_EMBED + cs:D_EMBED + ce]).astype(bf16),
            "wv": np.ascontiguousarray(Wqkv[:, 2 * D_EMBED + cs:2 * D_EMBED + ce]).astype(bf16),
            "wr": np.ascontiguousarray(Wr[:, cs:ce]).astype(bf16),
            "wo": np.ascontiguousarray(Wo[cs:ce, :]),
            "bias_w": np.ascontiguousarray(r_w_bias[n0:n0 + NH_LOC].reshape(DH, 1)),
            "bias_r": np.ascontiguousarray(r_r_bias[n0:n0 + NH_LOC].reshape(DH, 1)),
        })
    return in_maps


def kernel(w, r, r_w_bias, r_r_bias, attn_mask, mems, Wqkv, Wr, Wo):
    from concourse.bass_utils import run_bass_kernel_spmd

    nc = _get_program()
    in_maps = _prep_inputs(w, r, r_w_bias, r_r_bias, attn_mask, mems, Wqkv, Wr, Wo)
    res = run_bass_kernel_spmd(nc, in_maps, list(range(N_CORES)))
    # out per core: (b, i, e) bf16 partial; sum over cores (head groups)
    total = np.zeros((BSZ, TOT_LEN, D_EMBED), np.float32)
    for core in range(N_CORES):
        total += res.results[core]["out"].astype(np.float32)
    return np.ascontiguousarray(total.transpose(1, 0, 2))  # (i, b, e)
